# revision 11
# baseline (speedup 1.0000x reference)
"""2-layer GCN (PyG GCNConv style) on 8 Trainium2 NeuronCores.

v4 strategy (node-parallel, per sharding hint):
  - Nodes range-sharded across 8 cores (R = N/8 rows each), bf16 matmuls,
    fp32 PSUM, norm factoring out = dis[dst] * sum_e (dis[src]*xw[src]).
  - Aggregation = one-hot S^T @ G matmuls over 128-edge chunks; G built by
    gpsimd.dma_gather (random row reads are DESCRIPTOR-RATE-bound, so the
    optimization currency is descriptor count + overlap):
      * self-loop chunk per dst tile -> plain sequential dma_start + identity
        matmul (no gather descriptors at all).
      * local edges (src in own shard) gathered from ag*_in BEFORE/DURING the
        AllGather windows; partial sums parked in SBUF (locacc), folded into
        the per-tile PSUM later via one identity matmul.
      * remote edges gathered after the AllGather, zero padding cost: pad
        slots hold idx=-1 (descriptors skipped by SWDGE) and the per-core
        true count is loaded into a register (value_load) per instruction.
  - AG2 split into two collectives (rows 0:Xa, Xa:R) so the first fires
    mid-phase-2 and is hidden under remaining compute.
  - Layer-2 source tables are remapped host-side to the (ag2a|ag2b) permuted
    layouts; all tables/counts are per-core input data, program is SPMD.
"""

import sys

for p in ("/opt/trn_rl_repo",):
    if p not in sys.path:
        sys.path.insert(0, p)

import numpy as np
import ml_dtypes

import concourse.bass as bass
import concourse.bacc as bacc
import concourse.mybir as mybir
import concourse.tile as tile
from concourse import bass_utils
from concourse.masks import make_identity

P = 128
NCORES = 8
GMAX = 8          # chunks per dma_gather instruction
BF16 = ml_dtypes.bfloat16

PAD_DSTL = 255.0  # is_equal(255, d) false for every d in 0..127


# ----------------------------------------------------------------------------
# Host-side preprocessing
# ----------------------------------------------------------------------------

def _group_tables(n_cores, ntiles, R, dst_tile, dst_loc, groups):
    """Pack per-(tile, group) chunk tables with uniform chunk counts.

    groups: dict gname -> (per-core list of (src_idx_array, dstl_array) per
    tile).  Returns layout dict + per-core (idx16, dstl, counts) arrays.
    """
    gnames = list(groups.keys())
    # uniform chunk count per group, max over tiles AND cores (>=1 instr)
    K = {}
    for g in gnames:
        mx = 1
        for c in range(n_cores):
            for t in range(ntiles):
                mx = max(mx, len(groups[g][c][t][0]))
        K[g] = (mx + P - 1) // P

    # column layout: per tile, groups in gnames order, contiguous chunks
    col = 0
    col0 = {}            # (t, g) -> starting chunk column
    instr_list = []      # emission order: (t, g, n_instr_for_group)
    ninstr = {}
    for t in range(ntiles):
        for g in gnames:
            col0[(t, g)] = col
            col += K[g]
            ninstr[(t, g)] = (K[g] + GMAX - 1) // GMAX
    total = col

    n_instr_total = sum(ninstr.values())
    per_core = []
    for c in range(n_cores):
        idx16 = np.full((P, total * 8), -1, np.int16)
        dl = np.full((P, total), PAD_DSTL, np.float32)
        counts = np.zeros((1, n_instr_total), np.int32)
        icnt = 0
        for t in range(ntiles):
            for g in gnames:
                vals, dstl_v = groups[g][c][t]
                n = len(vals)
                cstart = col0[(t, g)]
                # slot i (< n) -> chunk cstart + i//128, part i%128
                i = np.arange(n)
                chunk_col = cstart + i // P
                part = i % P
                idx16[part % 16, chunk_col * 8 + part // 16] = vals.astype(
                    np.int16)
                dl[part, chunk_col] = dstl_v
                # counts per instruction (split K[g] chunks into <=GMAX
                # pieces); must match the greedy slot placement above exactly
                rem = n
                ki = K[g]
                nin = ninstr[(t, g)]
                for ii in range(nin):
                    cap = min(GMAX, ki - ii * GMAX) * P
                    if rem > 0:
                        take = min(cap, rem)
                    else:
                        # empty instruction: plant one dummy valid idx at its
                        # first slot (dstl stays PAD -> contributes zero)
                        take = 1
                        base_chunk = cstart + ii * GMAX
                        idx16[0, base_chunk * 8] = 0
                    counts[0, icnt] = take
                    icnt += 1
                    rem -= take
                assert rem <= 0
        idx16[16:, :] = np.tile(idx16[0:16, :], (7, 1))
        per_core.append((idx16, dl.astype(BF16), counts))
    layout = dict(gnames=gnames, K=K, col0=col0, ninstr=ninstr, total=total,
                  n_instr_total=n_instr_total)
    return layout, per_core


def _preprocess(x, edge_index, n_cores, split_t=None):
    N = x.shape[0]
    R = N // n_cores
    assert R * n_cores == N
    ntiles = (R + P - 1) // P
    last_rows = R - (ntiles - 1) * P

    src = np.asarray(edge_index[0], np.int64)
    dst = np.asarray(edge_index[1], np.int64)

    deg = (np.bincount(dst, minlength=N) + 1).astype(np.float64)  # + self loop
    dis = (1.0 / np.sqrt(deg)).astype(np.float32)

    ha = N if N <= 32768 else (N + 1) // 2
    assert ha <= 32768 and (N - ha) <= 32768

    if split_t is None:
        split_t = ntiles // 2
    Xa = min(split_t * P, R)
    Xb = R - Xa
    assert n_cores * Xa <= 32768 and n_cores * Xb <= 32768

    core_id = dst // R
    dloc = dst - core_id * R
    tl = dloc // P
    dstl = (dloc - tl * P).astype(np.float32)
    src_core = src // R
    src_off = src - src_core * R

    is_loc = src_core == core_id

    def collect(sel, idxvals):
        """per-core, per-tile lists of (idx, dstl)."""
        out = []
        for c in range(n_cores):
            m = sel & (core_id == c)
            pc = []
            for t in range(ntiles):
                mt = m & (tl == t)
                pc.append((idxvals[mt], dstl[mt]))
            out.append(pc)
        return out

    # layer 1 groups
    g1 = {
        "loc": collect(is_loc, src_off),
        "ra": collect((~is_loc) & (src < ha), src),
        "rb": collect((~is_loc) & (src >= ha), src - ha),
    }
    lay1, tabs1 = _group_tables(n_cores, ntiles, R, tl, dstl, g1)

    # layer 2 groups (remote remapped to ag2a/ag2b permuted layouts)
    in_a = src_off < Xa
    idx_a = src_core * Xa + src_off
    idx_b = src_core * Xb + (src_off - Xa)
    g2 = {
        "loc": collect(is_loc, src_off),
        "ra": collect((~is_loc) & in_a, idx_a),
        "rb": collect((~is_loc) & (~in_a), idx_b),
    }
    lay2, tabs2 = _group_tables(n_cores, ntiles, R, tl, dstl, g2)

    packed = []
    RP = ntiles * P
    for c in range(n_cores):
        dis_c = np.zeros(RP, np.float32)
        dis_c[:R] = dis[c * R:(c + 1) * R]
        diso = np.ascontiguousarray(dis_c.reshape(ntiles, P).T)  # [P, ntiles]
        packed.append(dict(idx1=tabs1[c][0], dstl1=tabs1[c][1],
                           cnt1=tabs1[c][2], idx2=tabs2[c][0],
                           dstl2=tabs2[c][1], cnt2=tabs2[c][2], diso=diso))
    meta = dict(lay1=lay1, lay2=lay2, R=R, ntiles=ntiles, last_rows=last_rows,
                ha=ha, Xa=Xa, Xb=Xb, dis=dis)
    return packed, meta


# ----------------------------------------------------------------------------
# Device kernel builder
# ----------------------------------------------------------------------------

def build_nc(N, meta, F0, F1, F2, n_cores, has_b1=False, has_b2=False):
    f32 = mybir.dt.float32
    bf = mybir.dt.bfloat16
    i16 = mybir.dt.int16
    i32 = mybir.dt.int32
    AF = mybir.ActivationFunctionType
    R = meta["R"]
    ntiles = meta["ntiles"]
    last_rows = meta["last_rows"]
    ha = meta["ha"]
    Xa, Xb = meta["Xa"], meta["Xb"]
    lay1, lay2 = meta["lay1"], meta["lay2"]
    K0 = F0 // P
    H1 = F1 // P
    K2 = F1 // P
    RP = ntiles * P
    split_t = Xa // P if Xa % P == 0 else ntiles  # tile idx where AG2a fires

    nc = bacc.Bacc("TRN2", target_bir_lowering=False, debug=False,
                   num_devices=n_cores, num_swdge_queues=4)

    xT = nc.dram_tensor("xT", [F0, R], bf, kind="ExternalInput").ap()
    idx1_d = nc.dram_tensor("idx1", [P, lay1["total"] * 8], i16,
                            kind="ExternalInput").ap()
    dstl1_d = nc.dram_tensor("dstl1", [P, lay1["total"]], bf,
                             kind="ExternalInput").ap()
    cnt1_d = nc.dram_tensor("cnt1", [1, lay1["n_instr_total"]], i32,
                            kind="ExternalInput").ap()
    idx2_d = nc.dram_tensor("idx2", [P, lay2["total"] * 8], i16,
                            kind="ExternalInput").ap()
    dstl2_d = nc.dram_tensor("dstl2", [P, lay2["total"]], bf,
                             kind="ExternalInput").ap()
    cnt2_d = nc.dram_tensor("cnt2", [1, lay2["n_instr_total"]], i32,
                            kind="ExternalInput").ap()
    diso_d = nc.dram_tensor("diso", [P, ntiles], f32,
                            kind="ExternalInput").ap()
    W1_d = nc.dram_tensor("W1", [F0, F1], bf, kind="ExternalInput").ap()
    W2_d = nc.dram_tensor("W2", [F1, F2], bf, kind="ExternalInput").ap()
    if has_b1:
        b1_d = nc.dram_tensor("b1", [F1], bf, kind="ExternalInput").ap()
    if has_b2:
        b2_d = nc.dram_tensor("b2", [F2], bf, kind="ExternalInput").ap()
    if has_b1 or has_b2:
        invd_d = nc.dram_tensor("invd", [1, RP], bf, kind="ExternalInput").ap()
    out_d = nc.dram_tensor("out", [R, F2], f32, kind="ExternalOutput").ap()

    rg = [list(range(n_cores))]

    with tile.TileContext(nc) as tc:
        with (
            tc.tile_pool(name="dram", bufs=1, space="DRAM") as dram,
            tc.tile_pool(name="const", bufs=1) as const,
        ):
            ag1_in = dram.tile([R, F1], bf)
            ag1_out = dram.tile([N, F1], bf, addr_space="Shared")
            ag2_in = dram.tile([R, F2], bf)
            ag2a_out = dram.tile([n_cores * Xa, F2], bf, addr_space="Shared")
            if Xb > 0:
                ag2b_out = dram.tile([n_cores * Xb, F2], bf,
                                     addr_space="Shared")

            w1_sb = const.tile([P, K0 * F1], bf)
            nc.sync.dma_start(
                out=w1_sb[:].rearrange("p (k f) -> p k f", k=K0),
                in_=W1_d.rearrange("(k p) f -> p k f", p=P))
            w2_sb = const.tile([P, K2 * F2], bf)
            nc.sync.dma_start(
                out=w2_sb[:].rearrange("p (k f) -> p k f", k=K2),
                in_=W2_d.rearrange("(k p) f -> p k f", p=P))
            if has_b1:
                b1_row = const.tile([1, F1], bf)
                nc.sync.dma_start(out=b1_row[:, :], in_=b1_d[None, :])
            if has_b2:
                b2_row = const.tile([1, F2], bf)
                nc.sync.dma_start(out=b2_row[:, :], in_=b2_d[None, :])
            if has_b1 or has_b2:
                invd_sb = const.tile([1, RP], bf)
                nc.sync.dma_start(out=invd_sb[:, :], in_=invd_d[:, :])

            iota_i = const.tile([P, P], mybir.dt.int32)
            nc.gpsimd.iota(iota_i[:], pattern=[[1, P]], base=0,
                           channel_multiplier=0)
            iota_bf = const.tile([P, P], bf)
            nc.vector.tensor_copy(out=iota_bf[:], in_=iota_i[:])
            ident = const.tile([P, P], bf)
            make_identity(nc, ident[:])

            idx1_sb = const.tile([P, lay1["total"] * 8], i16)
            nc.sync.dma_start(out=idx1_sb[:], in_=idx1_d[:])
            dstl1_sb = const.tile([P, lay1["total"]], bf)
            nc.sync.dma_start(out=dstl1_sb[:], in_=dstl1_d[:])
            cnt1_sb = const.tile([1, lay1["n_instr_total"]], i32)
            nc.sync.dma_start(out=cnt1_sb[:], in_=cnt1_d[:])
            idx2_sb = const.tile([P, lay2["total"] * 8], i16)
            nc.sync.dma_start(out=idx2_sb[:], in_=idx2_d[:])
            dstl2_sb = const.tile([P, lay2["total"]], bf)
            nc.sync.dma_start(out=dstl2_sb[:], in_=dstl2_d[:])
            cnt2_sb = const.tile([1, lay2["n_instr_total"]], i32)
            nc.sync.dma_start(out=cnt2_sb[:], in_=cnt2_d[:])
            diso_sb = const.tile([P, ntiles], f32)
            nc.sync.dma_start(out=diso_sb[:], in_=diso_d[:])

            h1T = const.tile([P, H1 * RP], bf)
            locacc1 = const.tile([P, ntiles * F1], bf)
            locacc2 = const.tile([P, ntiles * F2], bf)

            qrr = [0]
            icnt = [0]  # instruction counter per layer (reset between)
            cnt_regs = [nc.gpsimd.alloc_register(f"cntreg{i}")
                        for i in range(4)]
            creg = [0]

            def build_S(pool, dstl_sb, c0, kb, tag):
                S = pool.tile([P, kb * P], bf, tag=tag)
                nc.vector.scalar_tensor_tensor(
                    out=S[:].rearrange("p (k d) -> p k d", k=kb),
                    in0=dstl_sb[:, c0:c0 + kb].unsqueeze(2)
                        .broadcast_to([P, kb, P]),
                    scalar=1.0,
                    in1=iota_bf[:].unsqueeze(1).broadcast_to([P, kb, P]),
                    op0=mybir.AluOpType.mult,
                    op1=mybir.AluOpType.is_equal)
                return S

            def gather_group(G, Gcol0, idx_sb, cnt_sb, src_ap, c0, kgrp, F):
                """Emit gather instructions for one (tile, group)."""
                for ii in range(0, kgrp, GMAX):
                    kk = min(GMAX, kgrp - ii)
                    j = c0 + ii
                    cr = cnt_regs[creg[0]]
                    creg[0] = (creg[0] + 1) % len(cnt_regs)
                    nc.gpsimd.reg_load(cr, cnt_sb[0:1, icnt[0]:icnt[0] + 1])
                    icnt[0] += 1
                    o = Gcol0 + ii
                    nc.gpsimd.dma_gather(
                        out_ap=G[:, o * F:(o + kk) * F]
                            .rearrange("p (k f) -> p k f", k=kk),
                        in_ap=src_ap,
                        idxs_ap=idx_sb[:, 8 * j: 8 * (j + kk)],
                        num_idxs=kk * P,
                        num_idxs_reg=cr,
                        elem_size=F,
                        queue_num=qrr[0])
                    qrr[0] = (qrr[0] + 1) % 4

            # ---------------- phase 1: xw1 = dis * (x_c @ W1) ----------------
            with (
                tc.tile_pool(name="p1x", bufs=1) as p1x,
                tc.tile_pool(name="p1o", bufs=3) as p1o,
                tc.tile_pool(name="p1ps", bufs=2, space="PSUM") as p1ps,
            ):
                xk = []
                for k in range(K0):
                    xt = p1x.tile([P, R], bf, tag=f"xk{k}", name=f"xk{k}")
                    nc.sync.dma_start(out=xt[:],
                                      in_=xT[k * P:(k + 1) * P, :])
                    xk.append(xt)
                for m in range(ntiles):
                    rows = last_rows if m == ntiles - 1 else P
                    ps = p1ps.tile([P, F1], f32)
                    for k in range(K0):
                        nc.tensor.matmul(
                            out=ps[:rows, :],
                            lhsT=xk[k][:, m * P: m * P + rows],
                            rhs=w1_sb[:, k * F1:(k + 1) * F1],
                            start=(k == 0), stop=(k == K0 - 1))
                    os = p1o.tile([P, F1], bf)
                    nc.scalar.activation(out=os[:rows, :], in_=ps[:rows, :],
                                         func=AF.Copy,
                                         scale=diso_sb[:rows, m:m + 1])
                    nc.sync.dma_start(out=ag1_in[m * P: m * P + rows, :],
                                      in_=os[:rows, :])

            nc.gpsimd.collective_compute(
                "AllGather", mybir.AluOpType.bypass, replica_groups=rg,
                ins=[ag1_in[:].opt()], outs=[ag1_out[:].opt()])

            # ------- early L1 (overlaps AG1): self-loop + local-src partial
            #         sums per tile -> locacc1 -------
            K1 = lay1["K"]
            with (
                tc.tile_pool(name="eGs", bufs=3) as eGs,
                tc.tile_pool(name="eGl", bufs=3) as eGl,
                tc.tile_pool(name="eS", bufs=3) as eS,
                tc.tile_pool(name="eps", bufs=2, space="PSUM") as eps,
            ):
                kloc = K1["loc"]
                glring = []
                for z in range(3):  # ring of named bufs, memset once (NaN
                    gz = eGl.tile([P, kloc * F1], bf, tag=f"gl{z}",
                                  name=f"gl{z}")  # guard for skipped slots)
                    nc.vector.memset(gz[:], 0.0)
                    glring.append(gz)
                icnt[0] = 0
                for t in range(ntiles):
                    rows = last_rows if t == ntiles - 1 else P
                    gs = eGs.tile([P, F1], bf, tag="gs")
                    nc.sync.dma_start(out=gs[:rows, :],
                                      in_=ag1_in[t * P: t * P + rows, :])
                    gl = glring[t % 3]
                    gather_group(gl, 0, idx1_sb, cnt1_sb, ag1_in[:, :],
                                 lay1["col0"][(t, "loc")], kloc, F1)
                    Sl = build_S(eS, dstl1_sb, lay1["col0"][(t, "loc")],
                                 kloc, "sl")
                    psl = eps.tile([P, F1], f32, tag="psl")
                    nc.tensor.matmul(out=psl[:], lhsT=ident[:rows, :],
                                     rhs=gs[:rows, :], start=True, stop=False)
                    for ck in range(kloc):
                        nc.tensor.matmul(
                            out=psl[:],
                            lhsT=Sl[:, ck * P:(ck + 1) * P],
                            rhs=gl[:, ck * F1:(ck + 1) * F1],
                            start=False, stop=(ck == kloc - 1))
                    nc.scalar.activation(
                        out=locacc1[:, t * F1:(t + 1) * F1], in_=psl[:],
                        func=AF.Copy)
                    # skip remote instruction counter slots for this tile
                    icnt[0] += lay1["ninstr"][(t, "ra")] \
                        + lay1["ninstr"][(t, "rb")]

            # ------- phase 2+3: remote aggregation, relu, h1T, h1@W2 -------
            kra, krb = K1["ra"], K1["rb"]
            with (
                tc.tile_pool(name="p2g", bufs=3) as p2g,
                tc.tile_pool(name="p2s", bufs=3) as p2s,
                tc.tile_pool(name="p2h", bufs=3) as p2h,
                tc.tile_pool(name="p2o", bufs=3) as p2o,
                tc.tile_pool(name="p2ps", bufs=2, space="PSUM") as p2ps,
                tc.tile_pool(name="p2pt", bufs=2, space="PSUM") as p2pt,
                tc.tile_pool(name="p2p3", bufs=2, space="PSUM") as p2p3,
            ):
                gring = []
                for z in range(3):
                    gz = p2g.tile([P, (kra + krb) * F1], bf, tag=f"g{z}",
                                  name=f"g{z}")
                    nc.vector.memset(gz[:], 0.0)
                    gring.append(gz)
                icnt[0] = 0
                for t in range(ntiles):
                    rows = last_rows if t == ntiles - 1 else P
                    icnt[0] += lay1["ninstr"][(t, "loc")]
                    G = gring[t % 3]
                    gather_group(G, 0, idx1_sb, cnt1_sb, ag1_out[0:ha, :],
                                 lay1["col0"][(t, "ra")], kra, F1)
                    if ha < N:
                        gather_group(G, kra, idx1_sb, cnt1_sb,
                                     ag1_out[ha:N, :],
                                     lay1["col0"][(t, "rb")], krb, F1)
                    else:
                        icnt[0] += lay1["ninstr"][(t, "rb")]
                    # ra/rb chunk cols are contiguous: build S in one shot
                    S = build_S(p2s, dstl1_sb, lay1["col0"][(t, "ra")],
                                kra + krb, "s")
                    ps = p2ps.tile([P, F1], f32, tag="ps")
                    for ck in range(kra + krb):
                        nc.tensor.matmul(
                            out=ps[:],
                            lhsT=S[:, ck * P:(ck + 1) * P],
                            rhs=G[:, ck * F1:(ck + 1) * F1],
                            start=(ck == 0), stop=False)
                    nc.tensor.matmul(
                        out=ps[:], lhsT=ident[:],
                        rhs=locacc1[:, t * F1:(t + 1) * F1],
                        start=False, stop=not has_b1)
                    if has_b1:
                        nc.tensor.matmul(
                            out=ps[:], lhsT=invd_sb[:, t * P:(t + 1) * P],
                            rhs=b1_row[:], start=False, stop=True)
                    hm = p2h.tile([P, F1], bf, tag="hm")
                    nc.scalar.activation(out=hm[:], in_=ps[:],
                                         func=AF.Relu,
                                         scale=diso_sb[:, t:t + 1])
                    for h in range(H1):
                        pt = p2pt.tile([P, P], bf, tag="pt")
                        nc.tensor.transpose(
                            out=pt[:], in_=hm[:, h * P:(h + 1) * P],
                            identity=ident[:])
                        nc.scalar.activation(
                            out=h1T[:, h * RP + t * P: h * RP + (t + 1) * P],
                            in_=pt[:], func=AF.Copy)
                    p3 = p2p3.tile([P, F2], f32, tag="p3")
                    for k in range(K2):
                        nc.tensor.matmul(
                            out=p3[:rows, :],
                            lhsT=h1T[:, k * RP + t * P: k * RP + t * P + rows],
                            rhs=w2_sb[:, k * F2:(k + 1) * F2],
                            start=(k == 0), stop=(k == K2 - 1))
                    o3 = p2o.tile([P, F2], bf, tag="o3")
                    nc.scalar.activation(out=o3[:rows, :], in_=p3[:rows, :],
                                         func=AF.Copy,
                                         scale=diso_sb[:rows, t:t + 1])
                    nc.sync.dma_start(out=ag2_in[t * P: t * P + rows, :],
                                      in_=o3[:rows, :])
                    if t == split_t - 1 and Xa > 0:
                        nc.gpsimd.collective_compute(
                            "AllGather", mybir.AluOpType.bypass,
                            replica_groups=rg,
                            ins=[ag2_in[0:Xa, :].opt()],
                            outs=[ag2a_out[:].opt()])

            if Xb > 0:
                nc.gpsimd.collective_compute(
                    "AllGather", mybir.AluOpType.bypass, replica_groups=rg,
                    ins=[ag2_in[Xa:R, :].opt()], outs=[ag2b_out[:].opt()])

            # ------- early L2 (overlaps AG2): self + local partials -------
            K2g = lay2["K"]
            with (
                tc.tile_pool(name="fGs", bufs=3) as fGs,
                tc.tile_pool(name="fGl", bufs=3) as fGl,
                tc.tile_pool(name="fS", bufs=3) as fS,
                tc.tile_pool(name="fps", bufs=2, space="PSUM") as fps,
            ):
                kloc = K2g["loc"]
                gl2ring = []
                for z in range(3):
                    gz = fGl.tile([P, kloc * F2], bf, tag=f"gl2{z}",
                                  name=f"gl2{z}")
                    nc.vector.memset(gz[:], 0.0)
                    gl2ring.append(gz)
                icnt[0] = 0
                for t in range(ntiles):
                    rows = last_rows if t == ntiles - 1 else P
                    gs = fGs.tile([P, F2], bf, tag="gs2")
                    nc.sync.dma_start(out=gs[:rows, :],
                                      in_=ag2_in[t * P: t * P + rows, :])
                    gl = gl2ring[t % 3]
                    gather_group(gl, 0, idx2_sb, cnt2_sb, ag2_in[:, :],
                                 lay2["col0"][(t, "loc")], kloc, F2)
                    Sl = build_S(fS, dstl2_sb, lay2["col0"][(t, "loc")],
                                 kloc, "sl2")
                    psl = fps.tile([P, F2], f32, tag="psl2")
                    nc.tensor.matmul(out=psl[:], lhsT=ident[:rows, :],
                                     rhs=gs[:rows, :], start=True, stop=False)
                    for ck in range(kloc):
                        nc.tensor.matmul(
                            out=psl[:],
                            lhsT=Sl[:, ck * P:(ck + 1) * P],
                            rhs=gl[:, ck * F2:(ck + 1) * F2],
                            start=False, stop=(ck == kloc - 1))
                    nc.scalar.activation(
                        out=locacc2[:, t * F2:(t + 1) * F2], in_=psl[:],
                        func=AF.Copy)
                    icnt[0] += lay2["ninstr"][(t, "ra")] \
                        + lay2["ninstr"][(t, "rb")]

            # ------- phase 4: remote aggregation layer 2, fp32 out -------
            kra2, krb2 = K2g["ra"], K2g["rb"]
            with (
                tc.tile_pool(name="p4g", bufs=3) as p4g,
                tc.tile_pool(name="p4s", bufs=3) as p4s,
                tc.tile_pool(name="p4o", bufs=3) as p4o,
                tc.tile_pool(name="p4ps", bufs=3, space="PSUM") as p4ps,
            ):
                g4ring = []
                for z in range(3):
                    gz = p4g.tile([P, (kra2 + krb2) * F2], bf, tag=f"g4{z}",
                                  name=f"g4{z}")
                    nc.vector.memset(gz[:], 0.0)
                    g4ring.append(gz)
                icnt[0] = 0
                for t in range(ntiles):
                    rows = last_rows if t == ntiles - 1 else P
                    icnt[0] += lay2["ninstr"][(t, "loc")]
                    G = g4ring[t % 3]
                    gather_group(G, 0, idx2_sb, cnt2_sb, ag2a_out[:, :],
                                 lay2["col0"][(t, "ra")], kra2, F2)
                    if Xb > 0:
                        gather_group(G, kra2, idx2_sb, cnt2_sb,
                                     ag2b_out[:, :],
                                     lay2["col0"][(t, "rb")], krb2, F2)
                    else:
                        icnt[0] += lay2["ninstr"][(t, "rb")]
                    S = build_S(p4s, dstl2_sb, lay2["col0"][(t, "ra")],
                                kra2 + krb2, "s4")
                    ps = p4ps.tile([P, F2], f32, tag="ps4")
                    for ck in range(kra2 + krb2):
                        nc.tensor.matmul(
                            out=ps[:],
                            lhsT=S[:, ck * P:(ck + 1) * P],
                            rhs=G[:, ck * F2:(ck + 1) * F2],
                            start=(ck == 0), stop=False)
                    nc.tensor.matmul(
                        out=ps[:], lhsT=ident[:],
                        rhs=locacc2[:, t * F2:(t + 1) * F2],
                        start=False, stop=not has_b2)
                    if has_b2:
                        nc.tensor.matmul(
                            out=ps[:], lhsT=invd_sb[:, t * P:(t + 1) * P],
                            rhs=b2_row[:], start=False, stop=True)
                    ot = p4o.tile([P, F2], f32, tag="ot")
                    nc.scalar.activation(out=ot[:rows, :], in_=ps[:rows, :],
                                         func=AF.Copy,
                                         scale=diso_sb[:rows, t:t + 1])
                    nc.sync.dma_start(out=out_d[t * P: t * P + rows, :],
                                      in_=ot[:rows, :])

    nc.compile()
    return nc


# ----------------------------------------------------------------------------
# Public entry point
# ----------------------------------------------------------------------------

LAST_EXEC_NS = None
LAST_RESULTS = None


def kernel(x, edge_index, W1, b1, W2, b2, _trace=False, _tmpdir=None):
    global LAST_EXEC_NS, LAST_RESULTS
    x = np.asarray(x, np.float32)
    edge_index = np.asarray(edge_index)
    W1 = np.asarray(W1, np.float32)
    b1 = np.asarray(b1, np.float32)
    W2 = np.asarray(W2, np.float32)
    b2 = np.asarray(b2, np.float32)
    N, F0 = x.shape
    F1 = W1.shape[1]
    F2 = W2.shape[1]
    has_b1 = bool(np.any(b1 != 0))
    has_b2 = bool(np.any(b2 != 0))

    packed, meta = _preprocess(x, edge_index, NCORES)
    nc = build_nc(N, meta, F0, F1, F2, NCORES,
                  has_b1=has_b1, has_b2=has_b2)

    R = meta["R"]
    RP = meta["ntiles"] * P
    dis = meta["dis"]
    in_maps = []
    for c in range(NCORES):
        pc = packed[c]
        xT_c = np.ascontiguousarray(x[c * R:(c + 1) * R].T).astype(BF16)
        im = {
            "xT": xT_c, "idx1": pc["idx1"], "dstl1": pc["dstl1"],
            "cnt1": pc["cnt1"], "idx2": pc["idx2"], "dstl2": pc["dstl2"],
            "cnt2": pc["cnt2"], "diso": pc["diso"],
            "W1": W1.astype(BF16), "W2": W2.astype(BF16),
        }
        if has_b1:
            im["b1"] = b1.astype(BF16)
        if has_b2:
            im["b2"] = b2.astype(BF16)
        if has_b1 or has_b2:
            invd = np.zeros((1, RP), np.float32)
            invd[0, :R] = 1.0 / dis[c * R:(c + 1) * R]
            im["invd"] = invd.astype(BF16)
        in_maps.append(im)

    res = bass_utils.run_bass_kernel_spmd(
        nc, in_maps, core_ids=list(range(NCORES)), trace=_trace,
        tmpdir=_tmpdir)
    LAST_EXEC_NS = res.exec_time_ns
    LAST_RESULTS = res
    out = np.concatenate([res.results[c]["out"] for c in range(NCORES)],
                         axis=0)
    return out.astype(np.float32)
